# revision 40
# baseline (speedup 1.0000x reference)
"""GQA (grouped-query attention) Trainium2 kernel, 8-core SPMD.

Problem: B=4, T=2048, d_model=2048, 32 Q heads, 8 KV heads, d_k=64, causal.
Sharding: core = (batch b, half-of-KV-heads h): 8 cores = 4 batches x 2 halves.
Each core computes its 4 KV heads (16 Q heads) for its batch and the partial
output o_half @ Wo_half (row-parallel Wo); host sums the two halves per batch
and adds bo.

Device-side design (per core):
  - x^T (pre-transposed + bf16-cast on host) resident in SBUF.
  - k^T = Wk^T x^T (stored twice: original and partition-half-swapped, so
    every q head finds its kv head's k^T on its own partition half) and
    v = x Wv projections first (v gets a ones column appended so the PV
    matmul also produces the softmax denominator).
  - Heads are processed in pairs (2i, 2i+1): head 2i lives on SBUF
    partitions 0-63, head 2i+1 on 64-127, so their score matmuls
    (K=d_k=64 contraction) occupy disjoint PE row-groups and execute
    CONCURRENTLY (~2x score throughput). Both write halves of one
    [128, 1024] two-bank PSUM tile, consumed by a single wide exp on the
    scalar engine (halves the ACT per-instruction overhead).
  - Causal masking by multiplying the diagonal-crossing region (width
    128*(di+1) only) with a sliding window of a precomputed 0/1 mask.
  - Skewed PV: chunk ck-1's probabilities feed the PV matmuls while
    chunk ck's exp runs; both heads of a pair share the V weights.
  - Softmax division: per head, DVE reciprocal of the sums row (read
    straight from PSUM), GpSimd partition_broadcast, DVE multiply into
    the bf16 o^T tile.
  - Software pipelining: a global queue of O-projection (tile j-1 and
    older) and Q-projection (tile j+1) matmul groups is drained at a
    uniform rate between attention chunks so the TensorEngine stays
    busy while ACT computes exp, in every j-tile including the last.
  - Causality skips fully-masked (tk > all tq) score/PV tiles entirely.
"""

import numpy as np
import ml_dtypes
from contextlib import ExitStack

B, T, D = 4, 2048, 2048
NKV, NREP, DK = 8, 4, 64
HALF_KV = 4                  # kv heads per core
NQH = HALF_KV * NREP         # 16 q heads per core
NPAIR = NQH // 2             # 8 head pairs per core
QD = NQH * DK                # 1024 q dims per core
KVD = HALF_KV * DK           # 256 kv dims per core
NCORES = 8
CD = D // 128                # 16 contraction chunks over d_model
CT = T // 128                # 16 token chunks of 128
TQ = 512                     # query tile width
NTQ = T // TQ                # 4 query tiles
SCALE = 1.0 / np.sqrt(DK)

BF16 = ml_dtypes.bfloat16

_cache = {}


def _body(ctx, tc, aps):
    import concourse.mybir as mybir
    from concourse.bass import ts, ds

    nc = tc.nc
    f32 = mybir.dt.float32
    bf16 = mybir.dt.bfloat16
    xT, Wq, bqv, Wk, bkv, Wv, bv, Wo, out = (
        aps["xT"], aps["Wq"], aps["bq"], aps["Wk"], aps["bk"], aps["Wv"],
        aps["bv"], aps["Wo"], aps["out"])

    # ---- pools ----------------------------------------------------------
    rp = ctx.enter_context(tc.tile_pool(name="res", bufs=1))
    qp = ctx.enter_context(tc.tile_pool(name="qt", bufs=2))
    op = ctx.enter_context(tc.tile_pool(name="ot", bufs=2))
    ptp = ctx.enter_context(tc.tile_pool(name="pt", bufs=3))
    dvp = ctx.enter_context(tc.tile_pool(name="dv", bufs=2))
    wp = ctx.enter_context(tc.tile_pool(name="wk", bufs=2))
    # PSUM budget (8 banks): ss pair tiles 2x2 + a shared 4-deep ring
    # for the o65 accumulators and the projection-filler tiles (they are
    # allocated through one FIFO pump stream, so the ring stays ordered)
    pps = ctx.enter_context(tc.tile_pool(name="ps2", bufs=2, space="PSUM"))
    po = ctx.enter_context(tc.tile_pool(name="po", bufs=4, space="PSUM"))
    ppj = po

    # ---- resident tiles -------------------------------------------------
    xT_sb = rp.tile([128, CD, T], bf16, tag="xT")           # 64 KiB/part
    Wq_sb = rp.tile([128, CD, QD], bf16, tag="Wq")          # 32 KiB/part
    Wo_sb = rp.tile([128, QD // 128, D], bf16, tag="Wo")    # 32 KiB/part
    # kT2 blocks 0-1: kv head kv at partitions (kv%2)*64, col block kv//2.
    # blocks 2-3: the partition-half-swapped copy of blocks 0-1.
    kT2_sb = rp.tile([128, 4, T], bf16, tag="kT")           # 16 KiB/part
    v_sb = rp.tile([128, CT, HALF_KV, DK + 1], bf16, tag="v")
    bq_sb = rp.tile([128, QD // 128], f32, tag="bq")
    bk_sb = rp.tile([128, KVD // 128], f32, tag="bk")
    bv_sb = rp.tile([1, KVD], bf16, tag="bv")
    ones_b = rp.tile([1, 128], bf16, tag="ones_b")
    # causal triangle for the single 128x128 diagonal-crossing block:
    # tri[p, g] = (g >= p)
    tri = rp.tile([128, 128], bf16, tag="masks")
    # Wk/Wv share the qT tag: their slots are recycled into qT buffers
    # once the K/V projections are done.
    Wk_sb = qp.tile([128, CD, KVD], bf16, tag="qT")
    Wv_sb = qp.tile([128, CD, KVD], bf16, tag="qT")

    # DMA order matters: the K/V-projection inputs (xT, Wk, Wv) stream
    # chunk-interleaved on the sync queue, then Wq, and Wo last; the tiny
    # bias transfers issue in parallel from the idle gpsimd queue.
    for c in range(KVD // 128):
        nc.scalar.dma_start(bk_sb[:, c:c + 1], bkv[c, :].unsqueeze(-1))
    nc.scalar.dma_start(bv_sb[:, :], bv[:, :])
    for c in range(QD // 128):
        nc.scalar.dma_start(bq_sb[:, c:c + 1], bqv[c, :].unsqueeze(-1))
    for c in range(CD):
        nc.sync.dma_start(xT_sb[:, c, :], xT[c * 128:(c + 1) * 128, :])
        nc.gpsimd.dma_start(Wk_sb[:, c, :], Wk[c * 128:(c + 1) * 128, :])
        nc.gpsimd.dma_start(Wv_sb[:, c, :], Wv[c * 128:(c + 1) * 128, :])
    for c in range(CD):
        nc.scalar.dma_start(Wq_sb[:, c, :], Wq[c * 128:(c + 1) * 128, :])
    for c in range(QD // 128):
        nc.sync.dma_start(Wo_sb[:, c, :], Wo[c * 128:(c + 1) * 128, :])

    nc.vector.memset(ones_b[:, :], 1.0)
    nc.vector.memset(v_sb[:, :, :, DK:DK + 1], 1.0)
    nc.vector.memset(tri[:, :], 1.0)
    nc.gpsimd.affine_select(
        out=tri[:, :], in_=tri[:, :],
        compare_op=mybir.AluOpType.is_ge, fill=0.0,
        base=0, pattern=[[1, 128]], channel_multiplier=-1)

    # ---- K^T projection: all 8 accumulation groups in one c loop (the
    # attention PSUM pools are idle at startup, so borrow their banks) so
    # the PE stays busy and HAM-warm while the input DMA streams in -----
    kpA = pps.tile([128, 2, TQ], f32, tag="ss", name="kpA")
    kpB = pps.tile([128, 2, TQ], f32, tag="ss", name="kpB")
    kpC = po.tile([128, TQ], f32, tag="o65", name="kpC")
    kpD = po.tile([128, TQ], f32, tag="o65", name="kpD")
    kpE = po.tile([128, TQ], f32, tag="o65", name="kpE")
    kpF = po.tile([128, TQ], f32, tag="o65", name="kpF")
    kgrp = {(0, 0): kpA[:, 0, :], (0, 1): kpA[:, 1, :],
            (1, 0): kpB[:, 0, :], (1, 1): kpB[:, 1, :],
            (0, 2): kpC[:, :], (1, 2): kpD[:, :],
            (0, 3): kpE[:, :], (1, 3): kpF[:, :]}
    for c in range(CD):
        for (m, n), ap in kgrp.items():
            nc.tensor.matmul(ap, Wk_sb[:, c, ts(m, 128)],
                             xT_sb[:, c, ts(n, TQ)],
                             start=(c == 0), stop=(c == CD - 1))
    for (m, n), ap in kgrp.items():
        nc.vector.tensor_scalar_add(kT2_sb[:, m, ts(n, TQ)], ap,
                                    bk_sb[:, m:m + 1])
        # half-swapped copy (blocks 2-3), biases already included
        nc.vector.tensor_copy(kT2_sb[0:64, 2 + m, ts(n, TQ)],
                              kT2_sb[64:128, m, ts(n, TQ)])
        nc.vector.tensor_copy(kT2_sb[64:128, 2 + m, ts(n, TQ)],
                              kT2_sb[0:64, m, ts(n, TQ)])

    # ---- V projection (normal layout): the first half (the chunks the
    # first two j-loops consume) runs upfront; mt 8-15 flow in as fillers
    def vproj_body(mt, ps):
        for c in range(CD):
            nc.tensor.matmul(ps[:, 0:KVD],
                             xT_sb[:, c, ts(mt, 128)],
                             Wv_sb[:, c, :],
                             start=(c == 0), stop=False)
            if c < CD - 1:
                yield
        nc.tensor.matmul(ps[:, 0:KVD], ones_b[:, :], bv_sb[:, :],
                         start=False, stop=True)
        nc.vector.tensor_copy(v_sb[:, mt, :, 0:DK],
                              ps[:, 0:KVD].rearrange("p (h d) -> p h d",
                                                     h=HALF_KV))
        yield

    for mt in range(CT // 4):
        ps2 = pps.tile([128, 2, TQ], f32, tag="ss", name=f"vp{mt}")
        for _ in vproj_body(mt, ps2[:, 0, :]):
            pass

    # ---- projection matmul-group generators -----------------------------
    def qproj_group(jj, qT_tile, m, pool):
        # generator: one PE matmul per next() so it can be paced as filler
        ps = pool.tile([128, TQ], f32, tag="ss" if pool is pps else "o65",
                       name=f"q{jj}_{m}")
        for c in range(CD):
            nc.tensor.matmul(ps[:, :],
                             Wq_sb[:, c, ts(m, 128)],
                             xT_sb[:, c, ds(jj * TQ, TQ)],
                             start=(c == 0), stop=(c == CD - 1))
            if c < CD - 1:
                yield
        # eviction on the scalar engine (has slack whenever Q-proj fillers
        # run) to keep the DVE free for masks/divisions
        nc.scalar.activation(qT_tile[:, m, :], ps[:, :],
                             mybir.ActivationFunctionType.Identity,
                             bias=bq_sb[:, m:m + 1])
        yield

    def oproj_group(jj, oT_tile, mt, n, pool):
        ps = pool.tile([128, TQ], f32, tag="ss" if pool is pps else "o65",
                       name=f"o{jj}_{mt}_{n}")
        for c in range(QD // 128):
            nc.tensor.matmul(ps[:, :],
                             oT_tile[:, c, ts(mt, 128)],
                             Wo_sb[:, c, ts(n, TQ)],
                             start=(c == 0), stop=(c == QD // 128 - 1))
            if c < QD // 128 - 1:
                yield
        os_ = wp.tile([128, TQ], f32, tag="os", name=f"os{jj}_{mt}_{n}")
        nc.vector.tensor_copy(os_[:, :], ps[:, :])
        nc.sync.dma_start(
            out[ds(jj * TQ + mt * 128, 128), ts(n, TQ)], os_[:, :])
        yield

    # ---- pipelined main loop --------------------------------------------
    qT_tiles = {}
    oT_tiles = {}
    # prologue: q^T blocks 0-1 of tile 0 (pairs 0-1); the rest of tile 0
    # flows in as priority fillers inside loop 0
    qT_tiles[0] = qp.tile([128, QD // 128, TQ], bf16, tag="qT", name="qT_t0")
    for m in range(1):
        for _ in qproj_group(0, qT_tiles[0], m, pps):
            pass

    # global filler queue: (kind, key, generator). Entries are strictly
    # FIFO (generators from the shared single-buffer PSUM pool must never
    # interleave), so draining advances the head until the target is gone.
    fill_q = []
    for m in range(1, QD // 128):
        fill_q.append(("q", (0, m), qproj_group(0, qT_tiles[0], m, ppj)))
    def vproj_filler(mt):
        # allocate the PSUM tile lazily (at first pump) so the bufs=1
        # ring is claimed in strict FIFO order
        ps = ppj.tile([128, TQ], f32, tag="o65", name=f"vf{mt}")
        yield from vproj_body(mt, ps[:, :])

    for mt in range(CT // 4, CT):
        fill_q.append(("v", mt, vproj_filler(mt)))
    # Q0 rest, V rest, Q-proj 1-3, O-proj 0-2
    n_fillable = 7 * 16 + 12 * 17 + 3 * 128 + 3 * 128
    emitted = [0]
    suffix_cks = [0] * (NTQ + 1)
    for j in range(NTQ - 1, -1, -1):
        suffix_cks[j] = suffix_cks[j + 1] + NPAIR * (4 * j + 4)

    def pump(k):
        while k > 0 and fill_q:
            if next(fill_q[0][2], "done") == "done":
                fill_q.pop(0)
            else:
                k -= 1
                emitted[0] += 1

    def drain_kind(kind, key):
        while any(e[0] == kind and e[1] == key for e in fill_q):
            if next(fill_q[0][2], "done") == "done":
                fill_q.pop(0)
            else:
                emitted[0] += 1

    for j in range(NTQ):
        qT_sb = qT_tiles[j]
        oT_sb = op.tile([128, QD // 128, TQ], bf16, tag="oT")
        oT_tiles[j] = oT_sb
        # make the next q tile and the previous o tile's projection
        # available as filler work
        if j < NTQ - 1:
            qT_tiles[j + 1] = qp.tile([128, QD // 128, TQ], bf16, tag="qT",
                                      name=f"qT_t{j+1}")
            for m in range(QD // 128):
                fill_q.append(("q", (j + 1, m),
                               qproj_group(j + 1, qT_tiles[j + 1], m, ppj)))
        if j > 0:
            for mt in range(TQ // 128):
                for n in range(D // TQ):
                    fill_q.append(("o", j - 1,
                                   oproj_group(j - 1, oT_tiles[j - 1],
                                               mt, n, ppj)))

        nkeep = 4 * j + 4
        # the V chunks this loop's PV consumes must be emitted already
        for mt in range(CT // 4, min(CT, nkeep)):
            drain_kind("v", mt)
        rate = (n_fillable - emitted[0]) / suffix_cks[j]
        if j == 0:
            # overdrive: q tile 0's rest, the V fillers and q tile 1 must
            # all land during this (attention-light) loop
            rate = max(rate, 8.0)
        fill_acc = 0.0

        for i in range(NPAIR):
            # pair i reads q block i: its projection group must be emitted
            # (not just scheduled) before the pair's scores
            drain_kind("q", (j, i))
            kv = i // 2
            blkA = (kv // 2) if (kv % 2) == 0 else 2 + kv // 2
            blkB = (kv // 2) if (kv % 2) == 1 else 2 + kv // 2
            qA = qT_sb[0:64, i, :]
            qB = qT_sb[64:128, i, :]
            o65A = po.tile([65, TQ], f32, tag="o65", name=f"oA{j}_{i}")
            o65B = po.tile([65, TQ], f32, tag="o65", name=f"oB{j}_{i}")
            # skew=2 software pipeline: chunk ck's scores+exp are issued
            # two iterations before its PV consumes them, so the PV's
            # semaphore (exp done) is satisfied long before the tensor
            # engine reaches it and the FIFO never stalls on ACT latency.
            SKEW = 2
            pts = {}

            def c0_of(ck):
                return max(0, 128 * (ck - 4 * j))

            def emit_scores(ck):
                c0 = c0_of(ck)
                ss2 = pps.tile([128, 2, TQ], f32, tag="ss",
                               name=f"ss{j}_{i}_{ck}")
                nc.tensor.matmul(ss2[:, 0, c0:TQ],
                                 kT2_sb[0:64, blkA, ts(ck, 128)],
                                 qA[:, c0:TQ], start=True, stop=True)
                nc.tensor.matmul(ss2[:, 1, c0:TQ],
                                 kT2_sb[64:128, blkB, ts(ck, 128)],
                                 qB[:, c0:TQ], start=True, stop=True)
                pT2 = ptp.tile([128, 2, TQ], bf16, tag="pT",
                               name=f"pT{j}_{i}_{ck}")
                nc.scalar.activation(pT2[:, :, c0:TQ], ss2[:, :, c0:TQ],
                                     mybir.ActivationFunctionType.Exp,
                                     scale=SCALE)
                if ck - 4 * j >= 0:
                    # triangle mask on the diagonal-crossing block of both
                    # heads in one op (mask broadcast along the head dim)
                    nc.vector.tensor_mul(
                        pT2[:, :, c0:c0 + 128], pT2[:, :, c0:c0 + 128],
                        tri[:, :].unsqueeze(1).broadcast_to((128, 2, 128)))
                pts[ck] = pT2

            def emit_pv(ck):
                c0 = c0_of(ck)
                pT2 = pts.pop(ck)
                nc.tensor.matmul(o65A[:, c0:TQ], v_sb[:, ck, kv, :],
                                 pT2[:, 0, c0:TQ],
                                 start=(ck == 0), stop=(ck == nkeep - 1))
                nc.tensor.matmul(o65B[:, c0:TQ], v_sb[:, ck, kv, :],
                                 pT2[:, 1, c0:TQ],
                                 start=(ck == 0), stop=(ck == nkeep - 1))

            for ck in range(nkeep + SKEW):
                if ck < nkeep:
                    emit_scores(ck)
                if ck >= SKEW:
                    emit_pv(ck - SKEW)
                fill_acc += rate
                k = int(fill_acc)
                fill_acc -= k
                pump(k)
            # softmax division per head: 1/sums (row 64) broadcast over
            # the 64 o^T rows, fused with the psum->sbuf eviction.
            # (the custom-DVE reciprocal cannot read PSUM: copy sums first)
            for h, o65 in ((0, o65A), (1, o65B)):
                srow = dvp.tile([1, TQ], f32, tag="sr", name=f"sr{j}_{i}_{h}")
                nc.vector.tensor_copy(srow[:, :], o65[64:65, :])
                rrow = dvp.tile([1, TQ], f32, tag="rr", name=f"rr{j}_{i}_{h}")
                nc.vector.reciprocal_approx_fast(rrow[:, :], srow[:, :])
                bcs = dvp.tile([64, TQ], f32, tag="bc", name=f"bc{j}_{i}_{h}")
                nc.gpsimd.partition_broadcast(bcs[:, :], rrow[:, :])
                nc.vector.tensor_mul(
                    oT_sb[h * 64:h * 64 + 64, i, :], o65[0:64, :], bcs[:, :])
        # the oT ring slot reused two loops out must not have stale
        # readers: drain the previous tile's O-projection now
        if j > 0:
            drain_kind("o", j - 1)

    # drain remaining o-projection fillers, then the last tile's
    # O-projection (back-to-back, wide-PSUM pool so evictions overlap)
    while fill_q:
        pump(1 << 30)
    for mt in range(TQ // 128):
        for n in range(D // TQ):
            for _ in oproj_group(NTQ - 1, oT_tiles[NTQ - 1], mt, n, pps):
                pass


def _build():
    import concourse.mybir as mybir
    import concourse.tile as tile
    from concourse import bacc

    nc = bacc.Bacc("TRN2", target_bir_lowering=False, debug=False,
                   num_devices=NCORES)
    f32, bf16 = mybir.dt.float32, mybir.dt.bfloat16
    aps = {
        "xT": nc.dram_tensor("xT", (D, T), bf16, kind="ExternalInput").ap(),
        "Wq": nc.dram_tensor("Wq", (D, QD), bf16, kind="ExternalInput").ap(),
        "bq": nc.dram_tensor("bq", (QD // 128, 128), f32,
                             kind="ExternalInput").ap(),
        "Wk": nc.dram_tensor("Wk", (D, KVD), bf16, kind="ExternalInput").ap(),
        "bk": nc.dram_tensor("bk", (KVD // 128, 128), f32,
                             kind="ExternalInput").ap(),
        "Wv": nc.dram_tensor("Wv", (D, KVD), bf16, kind="ExternalInput").ap(),
        "bv": nc.dram_tensor("bv", (1, KVD), bf16, kind="ExternalInput").ap(),
        "Wo": nc.dram_tensor("Wo", (QD, D), bf16, kind="ExternalInput").ap(),
        "out": nc.dram_tensor("out", (T, D), f32, kind="ExternalOutput").ap(),
    }
    with tile.TileContext(nc) as tc:
        with ExitStack() as ctx:
            _body(ctx, tc, aps)
    nc.compile()
    return nc


def _get_nc():
    if "nc" not in _cache:
        _cache["nc"] = _build()
    return _cache["nc"]


def kernel(x, Wq, bq, Wk, bk, Wv, bv, Wo, bo, **_):
    from concourse.bass_utils import run_bass_kernel_spmd

    x = np.asarray(x, np.float32)
    in_maps = []
    for core in range(NCORES):
        b, h = core // 2, core % 2
        in_maps.append({
            "xT": np.ascontiguousarray(np.asarray(x[b]).T).astype(BF16),
            "Wq": np.asarray(Wq[:, h * QD:(h + 1) * QD], np.float32).astype(BF16),
            "bq": np.asarray(bq[h * QD:(h + 1) * QD], np.float32).reshape(
                QD // 128, 128),
            "Wk": np.asarray(Wk[:, h * KVD:(h + 1) * KVD], np.float32).astype(BF16),
            "bk": np.asarray(bk[h * KVD:(h + 1) * KVD], np.float32).reshape(
                KVD // 128, 128),
            "Wv": np.asarray(Wv[:, h * KVD:(h + 1) * KVD], np.float32).astype(BF16),
            "bv": np.asarray(bv[h * KVD:(h + 1) * KVD], np.float32).reshape(
                1, KVD).astype(BF16),
            "Wo": np.asarray(Wo[h * QD:(h + 1) * QD, :], np.float32).astype(BF16),
        })
    nc = _get_nc()
    res = run_bass_kernel_spmd(nc, in_maps, core_ids=list(range(NCORES)))
    bo = np.asarray(bo, np.float32)
    outs = [np.asarray(res.results[c]["out"], np.float32)
            for c in range(NCORES)]
    return np.stack([outs[2 * b] + outs[2 * b + 1] + bo
                     for b in range(B)], axis=0)


# revision 41
# speedup vs baseline: 1.0271x; 1.0271x over previous
"""GQA (grouped-query attention) Trainium2 kernel, 8-core SPMD.

Problem: B=4, T=2048, d_model=2048, 32 Q heads, 8 KV heads, d_k=64, causal.
Sharding: core = (batch b, half-of-KV-heads h): 8 cores = 4 batches x 2 halves.
Each core computes its 4 KV heads (16 Q heads) for its batch and the partial
output o_half @ Wo_half (row-parallel Wo); host sums the two halves per batch
and adds bo.

Device-side design (per core):
  - x^T (pre-transposed + bf16-cast on host) resident in SBUF.
  - k^T = Wk^T x^T (stored twice: original and partition-half-swapped, so
    every q head finds its kv head's k^T on its own partition half) and
    v = x Wv projections first (v gets a ones column appended so the PV
    matmul also produces the softmax denominator).
  - Heads are processed in pairs (2i, 2i+1): head 2i lives on SBUF
    partitions 0-63, head 2i+1 on 64-127, so their score matmuls
    (K=d_k=64 contraction) occupy disjoint PE row-groups and execute
    CONCURRENTLY (~2x score throughput). Both write halves of one
    [128, 1024] two-bank PSUM tile, consumed by a single wide exp on the
    scalar engine (halves the ACT per-instruction overhead).
  - Causal masking by multiplying the diagonal-crossing region (width
    128*(di+1) only) with a sliding window of a precomputed 0/1 mask.
  - Skewed PV: chunk ck-1's probabilities feed the PV matmuls while
    chunk ck's exp runs; both heads of a pair share the V weights.
  - Softmax division: per head, DVE reciprocal of the sums row (read
    straight from PSUM), GpSimd partition_broadcast, DVE multiply into
    the bf16 o^T tile.
  - Software pipelining: a global queue of O-projection (tile j-1 and
    older) and Q-projection (tile j+1) matmul groups is drained at a
    uniform rate between attention chunks so the TensorEngine stays
    busy while ACT computes exp, in every j-tile including the last.
  - Causality skips fully-masked (tk > all tq) score/PV tiles entirely.
"""

import numpy as np
import ml_dtypes
from contextlib import ExitStack

B, T, D = 4, 2048, 2048
NKV, NREP, DK = 8, 4, 64
HALF_KV = 4                  # kv heads per core
NQH = HALF_KV * NREP         # 16 q heads per core
NPAIR = NQH // 2             # 8 head pairs per core
QD = NQH * DK                # 1024 q dims per core
KVD = HALF_KV * DK           # 256 kv dims per core
NCORES = 8
CD = D // 128                # 16 contraction chunks over d_model
CT = T // 128                # 16 token chunks of 128
TQ = 512                     # query tile width
NTQ = T // TQ                # 4 query tiles
SCALE = 1.0 / np.sqrt(DK)

BF16 = ml_dtypes.bfloat16

_cache = {}


def _body(ctx, tc, aps):
    import concourse.mybir as mybir
    from concourse.bass import ts, ds

    nc = tc.nc
    f32 = mybir.dt.float32
    bf16 = mybir.dt.bfloat16
    xT, Wq, bqv, Wk, bkv, Wv, bv, Wo, out = (
        aps["xT"], aps["Wq"], aps["bq"], aps["Wk"], aps["bk"], aps["Wv"],
        aps["bv"], aps["Wo"], aps["out"])

    # ---- pools ----------------------------------------------------------
    rp = ctx.enter_context(tc.tile_pool(name="res", bufs=1))
    qp = ctx.enter_context(tc.tile_pool(name="qt", bufs=2))
    op = ctx.enter_context(tc.tile_pool(name="ot", bufs=2))
    ptp = ctx.enter_context(tc.tile_pool(name="pt", bufs=3))
    dvp = ctx.enter_context(tc.tile_pool(name="dv", bufs=2))
    wp = ctx.enter_context(tc.tile_pool(name="wk", bufs=2))
    # PSUM budget (8 banks): ss pair tiles 2x2 + a shared 4-deep ring
    # for the o65 accumulators and the projection-filler tiles (they are
    # allocated through one FIFO pump stream, so the ring stays ordered)
    pps = ctx.enter_context(tc.tile_pool(name="ps2", bufs=2, space="PSUM"))
    po = ctx.enter_context(tc.tile_pool(name="po", bufs=4, space="PSUM"))
    ppj = po

    # ---- resident tiles -------------------------------------------------
    xT_sb = rp.tile([128, CD, T], bf16, tag="xT")           # 64 KiB/part
    Wq_sb = rp.tile([128, CD, QD], bf16, tag="Wq")          # 32 KiB/part
    Wo_sb = rp.tile([128, QD // 128, D], bf16, tag="Wo")    # 32 KiB/part
    # kT2 blocks 0-1: kv head kv at partitions (kv%2)*64, col block kv//2.
    # blocks 2-3: the partition-half-swapped copy of blocks 0-1.
    kT2_sb = rp.tile([128, 4, T], bf16, tag="kT")           # 16 KiB/part
    v_sb = rp.tile([128, CT, HALF_KV, DK + 1], bf16, tag="v")
    bq_sb = rp.tile([128, QD // 128], f32, tag="bq")
    bk_sb = rp.tile([128, KVD // 128], f32, tag="bk")
    bv_sb = rp.tile([1, KVD], bf16, tag="bv")
    ones_b = rp.tile([1, 128], bf16, tag="ones_b")
    # causal triangle for the single 128x128 diagonal-crossing block:
    # tri[p, g] = (g >= p)
    tri = rp.tile([128, 128], bf16, tag="masks")
    # Wk/Wv share the qT tag: their slots are recycled into qT buffers
    # once the K/V projections are done.
    Wk_sb = qp.tile([128, CD, KVD], bf16, tag="qT")
    Wv_sb = qp.tile([128, CD, KVD], bf16, tag="qT")

    # DMA order matters: the K/V-projection inputs (xT, Wk, Wv) stream
    # chunk-interleaved on the sync queue, then Wq, and Wo last; the tiny
    # bias transfers issue in parallel from the idle gpsimd queue.
    for c in range(KVD // 128):
        nc.gpsimd.dma_start(bk_sb[:, c:c + 1], bkv[c, :].unsqueeze(-1))
    nc.gpsimd.dma_start(bv_sb[:, :], bv[:, :])
    for c in range(QD // 128):
        nc.gpsimd.dma_start(bq_sb[:, c:c + 1], bqv[c, :].unsqueeze(-1))
    for c in range(CD):
        nc.sync.dma_start(xT_sb[:, c, :], xT[c * 128:(c + 1) * 128, :])
        nc.sync.dma_start(Wk_sb[:, c, :], Wk[c * 128:(c + 1) * 128, :])
        nc.sync.dma_start(Wv_sb[:, c, :], Wv[c * 128:(c + 1) * 128, :])
    for c in range(CD):
        nc.sync.dma_start(Wq_sb[:, c, :], Wq[c * 128:(c + 1) * 128, :])
    for c in range(QD // 128):
        nc.sync.dma_start(Wo_sb[:, c, :], Wo[c * 128:(c + 1) * 128, :])

    nc.vector.memset(ones_b[:, :], 1.0)
    nc.vector.memset(v_sb[:, :, :, DK:DK + 1], 1.0)
    nc.vector.memset(tri[:, :], 1.0)
    nc.gpsimd.affine_select(
        out=tri[:, :], in_=tri[:, :],
        compare_op=mybir.AluOpType.is_ge, fill=0.0,
        base=0, pattern=[[1, 128]], channel_multiplier=-1)

    # ---- K^T projection: all 8 accumulation groups in one c loop (the
    # attention PSUM pools are idle at startup, so borrow their banks) so
    # the PE stays busy and HAM-warm while the input DMA streams in -----
    kpA = pps.tile([128, 2, TQ], f32, tag="ss", name="kpA")
    kpB = pps.tile([128, 2, TQ], f32, tag="ss", name="kpB")
    kpC = po.tile([128, TQ], f32, tag="o65", name="kpC")
    kpD = po.tile([128, TQ], f32, tag="o65", name="kpD")
    kpE = po.tile([128, TQ], f32, tag="o65", name="kpE")
    kpF = po.tile([128, TQ], f32, tag="o65", name="kpF")
    kgrp = {(0, 0): kpA[:, 0, :], (0, 1): kpA[:, 1, :],
            (1, 0): kpB[:, 0, :], (1, 1): kpB[:, 1, :],
            (0, 2): kpC[:, :], (1, 2): kpD[:, :],
            (0, 3): kpE[:, :], (1, 3): kpF[:, :]}
    for c in range(CD):
        for (m, n), ap in kgrp.items():
            nc.tensor.matmul(ap, Wk_sb[:, c, ts(m, 128)],
                             xT_sb[:, c, ts(n, TQ)],
                             start=(c == 0), stop=(c == CD - 1))
    for (m, n), ap in kgrp.items():
        nc.vector.tensor_scalar_add(kT2_sb[:, m, ts(n, TQ)], ap,
                                    bk_sb[:, m:m + 1])
        # half-swapped copy (blocks 2-3), biases already included
        nc.vector.tensor_copy(kT2_sb[0:64, 2 + m, ts(n, TQ)],
                              kT2_sb[64:128, m, ts(n, TQ)])
        nc.vector.tensor_copy(kT2_sb[64:128, 2 + m, ts(n, TQ)],
                              kT2_sb[0:64, m, ts(n, TQ)])

    # ---- V projection (normal layout): the first half (the chunks the
    # first two j-loops consume) runs upfront; mt 8-15 flow in as fillers
    def vproj_body(mt, ps):
        for c in range(CD):
            nc.tensor.matmul(ps[:, 0:KVD],
                             xT_sb[:, c, ts(mt, 128)],
                             Wv_sb[:, c, :],
                             start=(c == 0), stop=False)
            if c < CD - 1:
                yield
        nc.tensor.matmul(ps[:, 0:KVD], ones_b[:, :], bv_sb[:, :],
                         start=False, stop=True)
        nc.vector.tensor_copy(v_sb[:, mt, :, 0:DK],
                              ps[:, 0:KVD].rearrange("p (h d) -> p h d",
                                                     h=HALF_KV))
        yield

    for mt in range(CT // 4):
        ps2 = pps.tile([128, 2, TQ], f32, tag="ss", name=f"vp{mt}")
        for _ in vproj_body(mt, ps2[:, 0, :]):
            pass

    # ---- projection matmul-group generators -----------------------------
    def qproj_group(jj, qT_tile, m, pool):
        # generator: one PE matmul per next() so it can be paced as filler
        ps = pool.tile([128, TQ], f32, tag="ss" if pool is pps else "o65",
                       name=f"q{jj}_{m}")
        for c in range(CD):
            nc.tensor.matmul(ps[:, :],
                             Wq_sb[:, c, ts(m, 128)],
                             xT_sb[:, c, ds(jj * TQ, TQ)],
                             start=(c == 0), stop=(c == CD - 1))
            if c < CD - 1:
                yield
        # eviction on the scalar engine (has slack whenever Q-proj fillers
        # run) to keep the DVE free for masks/divisions
        nc.scalar.activation(qT_tile[:, m, :], ps[:, :],
                             mybir.ActivationFunctionType.Identity,
                             bias=bq_sb[:, m:m + 1])
        yield

    def oproj_group(jj, oT_tile, mt, n, pool):
        ps = pool.tile([128, TQ], f32, tag="ss" if pool is pps else "o65",
                       name=f"o{jj}_{mt}_{n}")
        for c in range(QD // 128):
            nc.tensor.matmul(ps[:, :],
                             oT_tile[:, c, ts(mt, 128)],
                             Wo_sb[:, c, ts(n, TQ)],
                             start=(c == 0), stop=(c == QD // 128 - 1))
            if c < QD // 128 - 1:
                yield
        os_ = wp.tile([128, TQ], f32, tag="os", name=f"os{jj}_{mt}_{n}")
        nc.vector.tensor_copy(os_[:, :], ps[:, :])
        nc.sync.dma_start(
            out[ds(jj * TQ + mt * 128, 128), ts(n, TQ)], os_[:, :])
        yield

    # ---- pipelined main loop --------------------------------------------
    qT_tiles = {}
    oT_tiles = {}
    # prologue: q^T blocks 0-1 of tile 0 (pairs 0-1); the rest of tile 0
    # flows in as priority fillers inside loop 0
    qT_tiles[0] = qp.tile([128, QD // 128, TQ], bf16, tag="qT", name="qT_t0")
    for m in range(1):
        for _ in qproj_group(0, qT_tiles[0], m, pps):
            pass

    # global filler queue: (kind, key, generator). Entries are strictly
    # FIFO (generators from the shared single-buffer PSUM pool must never
    # interleave), so draining advances the head until the target is gone.
    fill_q = []
    for m in range(1, QD // 128):
        fill_q.append(("q", (0, m), qproj_group(0, qT_tiles[0], m, ppj)))
    def vproj_filler(mt):
        # allocate the PSUM tile lazily (at first pump) so the bufs=1
        # ring is claimed in strict FIFO order
        ps = ppj.tile([128, TQ], f32, tag="o65", name=f"vf{mt}")
        yield from vproj_body(mt, ps[:, :])

    for mt in range(CT // 4, CT):
        fill_q.append(("v", mt, vproj_filler(mt)))
    # Q0 rest, V rest, Q-proj 1-3, O-proj 0-2
    n_fillable = 7 * 16 + 12 * 17 + 3 * 128 + 3 * 128
    emitted = [0]
    suffix_cks = [0] * (NTQ + 1)
    for j in range(NTQ - 1, -1, -1):
        suffix_cks[j] = suffix_cks[j + 1] + NPAIR * (4 * j + 4)

    def pump(k):
        while k > 0 and fill_q:
            if next(fill_q[0][2], "done") == "done":
                fill_q.pop(0)
            else:
                k -= 1
                emitted[0] += 1

    def drain_kind(kind, key):
        while any(e[0] == kind and e[1] == key for e in fill_q):
            if next(fill_q[0][2], "done") == "done":
                fill_q.pop(0)
            else:
                emitted[0] += 1

    for j in range(NTQ):
        qT_sb = qT_tiles[j]
        oT_sb = op.tile([128, QD // 128, TQ], bf16, tag="oT")
        oT_tiles[j] = oT_sb
        # make the next q tile and the previous o tile's projection
        # available as filler work
        if j < NTQ - 1:
            qT_tiles[j + 1] = qp.tile([128, QD // 128, TQ], bf16, tag="qT",
                                      name=f"qT_t{j+1}")
            for m in range(QD // 128):
                fill_q.append(("q", (j + 1, m),
                               qproj_group(j + 1, qT_tiles[j + 1], m, ppj)))
        if j > 0:
            for mt in range(TQ // 128):
                for n in range(D // TQ):
                    fill_q.append(("o", j - 1,
                                   oproj_group(j - 1, oT_tiles[j - 1],
                                               mt, n, ppj)))

        nkeep = 4 * j + 4
        # the V chunks this loop's PV consumes must be emitted already
        for mt in range(CT // 4, min(CT, nkeep)):
            drain_kind("v", mt)
        rate = (n_fillable - emitted[0]) / suffix_cks[j]
        if j == 0:
            # overdrive: q tile 0's rest, the V fillers and q tile 1 must
            # all land during this (attention-light) loop
            rate = max(rate, 8.0)
        fill_acc = 0.0

        for i in range(NPAIR):
            # pair i reads q block i: its projection group must be emitted
            # (not just scheduled) before the pair's scores
            drain_kind("q", (j, i))
            kv = i // 2
            blkA = (kv // 2) if (kv % 2) == 0 else 2 + kv // 2
            blkB = (kv // 2) if (kv % 2) == 1 else 2 + kv // 2
            qA = qT_sb[0:64, i, :]
            qB = qT_sb[64:128, i, :]
            o65A = po.tile([65, TQ], f32, tag="o65", name=f"oA{j}_{i}")
            o65B = po.tile([65, TQ], f32, tag="o65", name=f"oB{j}_{i}")
            # skew=2 software pipeline: chunk ck's scores+exp are issued
            # two iterations before its PV consumes them, so the PV's
            # semaphore (exp done) is satisfied long before the tensor
            # engine reaches it and the FIFO never stalls on ACT latency.
            SKEW = 2
            pts = {}

            def c0_of(ck):
                return max(0, 128 * (ck - 4 * j))

            def emit_scores(ck):
                c0 = c0_of(ck)
                ss2 = pps.tile([128, 2, TQ], f32, tag="ss",
                               name=f"ss{j}_{i}_{ck}")
                nc.tensor.matmul(ss2[:, 0, c0:TQ],
                                 kT2_sb[0:64, blkA, ts(ck, 128)],
                                 qA[:, c0:TQ], start=True, stop=True)
                nc.tensor.matmul(ss2[:, 1, c0:TQ],
                                 kT2_sb[64:128, blkB, ts(ck, 128)],
                                 qB[:, c0:TQ], start=True, stop=True)
                pT2 = ptp.tile([128, 2, TQ], bf16, tag="pT",
                               name=f"pT{j}_{i}_{ck}")
                nc.scalar.activation(pT2[:, :, c0:TQ], ss2[:, :, c0:TQ],
                                     mybir.ActivationFunctionType.Exp,
                                     scale=SCALE)
                if ck - 4 * j >= 0:
                    # triangle mask on the diagonal-crossing block of both
                    # heads in one op (mask broadcast along the head dim)
                    nc.vector.tensor_mul(
                        pT2[:, :, c0:c0 + 128], pT2[:, :, c0:c0 + 128],
                        tri[:, :].unsqueeze(1).broadcast_to((128, 2, 128)))
                pts[ck] = pT2

            def emit_pv(ck):
                c0 = c0_of(ck)
                pT2 = pts.pop(ck)
                nc.tensor.matmul(o65A[:, c0:TQ], v_sb[:, ck, kv, :],
                                 pT2[:, 0, c0:TQ],
                                 start=(ck == 0), stop=(ck == nkeep - 1))
                nc.tensor.matmul(o65B[:, c0:TQ], v_sb[:, ck, kv, :],
                                 pT2[:, 1, c0:TQ],
                                 start=(ck == 0), stop=(ck == nkeep - 1))

            for ck in range(nkeep + SKEW):
                if ck < nkeep:
                    emit_scores(ck)
                if ck >= SKEW:
                    emit_pv(ck - SKEW)
                fill_acc += rate
                k = int(fill_acc)
                fill_acc -= k
                pump(k)
            # softmax division per head: 1/sums (row 64) broadcast over
            # the 64 o^T rows, fused with the psum->sbuf eviction.
            # (the custom-DVE reciprocal cannot read PSUM: copy sums first)
            for h, o65 in ((0, o65A), (1, o65B)):
                srow = dvp.tile([1, TQ], f32, tag="sr", name=f"sr{j}_{i}_{h}")
                nc.vector.tensor_copy(srow[:, :], o65[64:65, :])
                rrow = dvp.tile([1, TQ], f32, tag="rr", name=f"rr{j}_{i}_{h}")
                nc.vector.reciprocal_approx_fast(rrow[:, :], srow[:, :])
                bcs = dvp.tile([64, TQ], f32, tag="bc", name=f"bc{j}_{i}_{h}")
                nc.gpsimd.partition_broadcast(bcs[:, :], rrow[:, :])
                nc.vector.tensor_mul(
                    oT_sb[h * 64:h * 64 + 64, i, :], o65[0:64, :], bcs[:, :])
        # the oT ring slot reused two loops out must not have stale
        # readers: drain the previous tile's O-projection now
        if j > 0:
            drain_kind("o", j - 1)

    # drain remaining o-projection fillers, then the last tile's
    # O-projection (back-to-back, wide-PSUM pool so evictions overlap)
    while fill_q:
        pump(1 << 30)
    for mt in range(TQ // 128):
        for n in range(D // TQ):
            for _ in oproj_group(NTQ - 1, oT_tiles[NTQ - 1], mt, n, pps):
                pass


def _build():
    import concourse.mybir as mybir
    import concourse.tile as tile
    from concourse import bacc

    nc = bacc.Bacc("TRN2", target_bir_lowering=False, debug=False,
                   num_devices=NCORES)
    f32, bf16 = mybir.dt.float32, mybir.dt.bfloat16
    aps = {
        "xT": nc.dram_tensor("xT", (D, T), bf16, kind="ExternalInput").ap(),
        "Wq": nc.dram_tensor("Wq", (D, QD), bf16, kind="ExternalInput").ap(),
        "bq": nc.dram_tensor("bq", (QD // 128, 128), f32,
                             kind="ExternalInput").ap(),
        "Wk": nc.dram_tensor("Wk", (D, KVD), bf16, kind="ExternalInput").ap(),
        "bk": nc.dram_tensor("bk", (KVD // 128, 128), f32,
                             kind="ExternalInput").ap(),
        "Wv": nc.dram_tensor("Wv", (D, KVD), bf16, kind="ExternalInput").ap(),
        "bv": nc.dram_tensor("bv", (1, KVD), bf16, kind="ExternalInput").ap(),
        "Wo": nc.dram_tensor("Wo", (QD, D), bf16, kind="ExternalInput").ap(),
        "out": nc.dram_tensor("out", (T, D), f32, kind="ExternalOutput").ap(),
    }
    with tile.TileContext(nc) as tc:
        with ExitStack() as ctx:
            _body(ctx, tc, aps)
    nc.compile()
    return nc


def _get_nc():
    if "nc" not in _cache:
        _cache["nc"] = _build()
    return _cache["nc"]


def kernel(x, Wq, bq, Wk, bk, Wv, bv, Wo, bo, **_):
    from concourse.bass_utils import run_bass_kernel_spmd

    x = np.asarray(x, np.float32)
    in_maps = []
    for core in range(NCORES):
        b, h = core // 2, core % 2
        in_maps.append({
            "xT": np.ascontiguousarray(np.asarray(x[b]).T).astype(BF16),
            "Wq": np.asarray(Wq[:, h * QD:(h + 1) * QD], np.float32).astype(BF16),
            "bq": np.asarray(bq[h * QD:(h + 1) * QD], np.float32).reshape(
                QD // 128, 128),
            "Wk": np.asarray(Wk[:, h * KVD:(h + 1) * KVD], np.float32).astype(BF16),
            "bk": np.asarray(bk[h * KVD:(h + 1) * KVD], np.float32).reshape(
                KVD // 128, 128),
            "Wv": np.asarray(Wv[:, h * KVD:(h + 1) * KVD], np.float32).astype(BF16),
            "bv": np.asarray(bv[h * KVD:(h + 1) * KVD], np.float32).reshape(
                1, KVD).astype(BF16),
            "Wo": np.asarray(Wo[h * QD:(h + 1) * QD, :], np.float32).astype(BF16),
        })
    nc = _get_nc()
    res = run_bass_kernel_spmd(nc, in_maps, core_ids=list(range(NCORES)))
    bo = np.asarray(bo, np.float32)
    outs = [np.asarray(res.results[c]["out"], np.float32)
            for c in range(NCORES)]
    return np.stack([outs[2 * b] + outs[2 * b + 1] + bo
                     for b in range(B)], axis=0)
